# revision 7
# baseline (speedup 1.0000x reference)
"""Trainium2 Bass kernel V2: GQA sliding-window attention, SBUF-resident bf16.

Problem: B=2, T=2048, D=3584, N=16 q-heads, K=8 kv-heads, H=256,
sliding window 1024, causal, soft-cap 50, query scale 0.0625, RoPE.

Sharding: 8 cores = 2 (batch) x 4 (head groups); each core: 4 q-heads,
2 kv-heads. Host sums the 4 partial out-projections per batch.

V2 design vs baseline:
  - All weights/intermediates bf16 (halves DMA + SBUF, full PE rate).
  - Zero DRAM round-trips: kT/qT/v/eT live in SBUF end-to-end.
  - V projected directly into [t, h] layout (stationary = x chunk), so
    no PE transposes and no copies.
  - SCALE/SOFT_CAP folded into q_w on host; RoPE drain = ACT copy
    (psum->bf16) + 6 bf16 DVE ops (4x mode) writing kT/qT in place.
  - Attention interleaves the two q-heads of each kv head to keep PE fed
    while the softmax chain (ACT tanh/exp, DVE mask-mul) runs.
  - Out-proj reads eT straight from SBUF, accumulating 8 matmuls in PSUM.
"""

import os
import sys

sys.path.insert(0, "/opt/trn_rl_repo")

import numpy as np

B, T, D = 2, 2048, 3584
NQ, NKV, H = 16, 8, 256
P = 128
DC = D // P                 # 28 contraction chunks
HEADS_PER_CORE = 4
KV_PER_CORE = 2
SOFT_CAP = 50.0
SCALE = 0.0625
WINDOW = 1024
BASE_FREQ = 10000.0
QTILE = 512
NQT = T // QTILE            # 4
ATILE = 256
NAT = T // ATILE            # 8
NKT = T // P                # 16

_NC_CACHE = {}
LAST_RESULTS = None


def _kt_list(at):
    """Valid k-tiles for q-block at (ATILE wide); mask index None = full."""
    Q0 = at * ATILE
    out = []
    for kt in range(NKT):
        K0 = kt * P
        if K0 > Q0 + ATILE - 1:
            continue
        if K0 + P - 1 <= Q0 - WINDOW:
            continue
        rel = K0 - Q0
        if rel >= 0:
            out.append((kt, rel // P))
        else:
            w = Q0 - K0 - WINDOW
            if -ATILE < w <= 0:
                out.append((kt, 2 + (-w) // P))
            else:
                out.append((kt, None))
    return out


def _make_masks():
    m = np.zeros((4, P, ATILE), np.float32)
    i = np.arange(P)[:, None]
    j = np.arange(ATILE)[None, :]
    for r in range(2):           # diag: allowed iff i <= j - rel
        m[r] = np.where(i <= j - r * P, 1.0, 0.0)
    for wi in range(2):          # window: allowed iff i > j - wi*128
        m[2 + wi] = np.where(i > j - wi * P, 1.0, 0.0)
    return m


def _build_nc():
    import concourse.bacc as bacc
    import concourse.mybir as mybir
    import concourse.tile as tile
    from concourse import bass_isa

    f32 = mybir.dt.float32
    bf16 = mybir.dt.bfloat16
    AF = mybir.ActivationFunctionType

    nc = bacc.Bacc()
    xT = nc.dram_tensor("xT", (P, DC, T), bf16, kind="ExternalInput")
    qw = nc.dram_tensor("qw", (HEADS_PER_CORE, P, DC, H), bf16,
                        kind="ExternalInput")
    kw = nc.dram_tensor("kw", (KV_PER_CORE, P, DC, H), bf16,
                        kind="ExternalInput")
    vw = nc.dram_tensor("vw", (P, DC, 2 * H), bf16, kind="ExternalInput")
    ow = nc.dram_tensor("ow", (2 * HEADS_PER_CORE, P, D), bf16,
                        kind="ExternalInput")
    rope = nc.dram_tensor("rope", (P, 2, T), bf16, kind="ExternalInput")
    msk = nc.dram_tensor("msk", (P, 4, 2 * ATILE), bf16,
                         kind="ExternalInput")
    out = nc.dram_tensor("out", (T, D), f32, kind="ExternalOutput")

    with tile.TileContext(nc) as tc:
        pers_cm = tc.tile_pool(name="pers", bufs=1)
        pers = pers_cm.__enter__()

        # Persistent SBUF state
        kT = [pers.tile([P, 2, T], bf16, tag=f"kT{i}", name=f"kT{i}")
              for i in range(KV_PER_CORE)]
        # q stored pair-packed per at-block: [hc, head-in-pair, ATILE] so
        # one 512-wide moving operand covers both heads of a kv pair, and
        # per-at tiles keep attention deps slice-accurate
        qP = [
            [pers.tile([P, 2, 2, ATILE], bf16, tag=f"qP{i}_{a}",
                       name=f"qP{i}_{a}") for a in range(NAT)]
            for i in range(KV_PER_CORE)
        ]
        vA = pers.tile([P, NKT, 2 * H], bf16, tag="vA", name="vA")
        masks_sb = pers.tile([P, 4, 2 * ATILE], bf16, tag="msk",
                             name="masks_sb")
        bias_m50 = pers.tile([P, 1], f32, tag="b50", name="bias_m50")
        nc.vector.memset(bias_m50[:], -SOFT_CAP)
        # warm the GpSimd ucode library during P1 so the first real
        # partition_all_reduce in attention doesn't stall ~10us on I$ load
        gwarm = pers.tile([P, 1], f32, tag="gw", name="gwarm")
        nc.gpsimd.partition_all_reduce(gwarm[:], bias_m50[:], P,
                                       bass_isa.ReduceOp.add)

        # ---------------- P1: projections + RoPE ----------------
        w_cm = tc.tile_pool(name="w", bufs=1)
        wpool = w_cm.__enter__()
        # rope table only lives through P1 — keep it in the P1 pool
        rope_sb = wpool.tile([P, 2, T], bf16, tag="rope", name="rope_sb")
        cos_a = rope_sb[:, 0]
        sin_a = rope_sb[:, 1]
        xt_cm = tc.tile_pool(name="xt", bufs=5)
        xpool = xt_cm.__enter__()
        tmp_cm = tc.tile_pool(name="tmp", bufs=2)
        tpool = tmp_cm.__enter__()
        ps1_cm = tc.tile_pool(name="ps1", bufs=1, space="PSUM")
        ps1 = ps1_cm.__enter__()

        for half in range(2):  # 0: k0,k1 + v(packed)   1: q0..q3
            # DMA issue order matters: every dma_start serializes on the
            # sync engine (~0.6us each), so issue the first-needed chunks
            # first: leading halves of the weights, then the first x tiles,
            # then the weight tails (and rope/masks, needed ~40us in).
            CSPLIT = 14

            if half == 0:
                # interleave weight-chunk and x-tile issue in contraction
                # order (geometric chunks): PE consumes ~380KB per d-group,
                # so arrival order must track the d loop or PE starves
                wk = [wpool.tile([P, DC, H], bf16, tag=f"w{j}",
                                 name=f"wk{j}") for j in range(2)]
                wv = wpool.tile([P, DC, 2 * H], bf16, tag="w2", name="wv")
                xt_pre = []

                def _xt_dma(d4):
                    xt = xpool.tile([P, 4, QTILE], bf16, tag="xt",
                                    name="xt")
                    nc.sync.dma_start(
                        xt[:], xT[:, 4 * d4 : 4 * d4 + 4, 0:QTILE]
                    )
                    xt_pre.append(xt)

                bounds = [0, 2, 6, 14, DC]
                for ci in range(4):
                    cs = slice(bounds[ci], bounds[ci + 1])
                    for j in range(2):
                        nc.sync.dma_start(wk[j][:, cs], kw[j][:, cs])
                    nc.sync.dma_start(wv[:, cs], vw[:, cs])
                    _xt_dma(ci)
                nc.sync.dma_start(rope_sb[:], rope[:])
                nc.sync.dma_start(masks_sb[:], msk[:])
            else:
                wq = [wq0]
                for j in range(1, 4):
                    wt = wpool.tile([P, DC, H], bf16, tag=f"w{j}",
                                    name=f"wq{j}")
                    nc.sync.dma_start(wt[:], qw[j])
                    wq.append(wt)

            for n in range(NQT):
                ns = slice(n * QTILE, (n + 1) * QTILE)
                if half == 0 and n == 2:
                    # qw0 prefetch on its own slot mid-half-0: no WAR gate,
                    # and the x-stream has DMA slack by now
                    wq0 = wpool.tile([P, DC, H], bf16, tag="w4", name="wq0")
                    nc.sync.dma_start(wq0[:], qw[0])
                if half == 0:
                    # k psums: tags t0..t3; v psums (per t-chunk): t4..t7
                    kps = [
                        [ps1.tile([P, QTILE], f32, tag=f"t{2 * j + hc}",
                                  name=f"kps{j}{hc}") for hc in range(2)]
                        for j in range(2)
                    ]
                    vps = [ps1.tile([P, 2 * H], f32, tag=f"t{4 + tci}",
                                    name=f"vps{tci}") for tci in range(4)]
                else:
                    qps = [
                        [ps1.tile([P, QTILE], f32, tag=f"t{2 * j + hc}",
                                  name=f"qps{j}{hc}") for hc in range(2)]
                        for j in range(4)
                    ]
                for dp in range(DC // 4):
                    if half == 0 and n == 0 and dp < 4:
                        xt = xt_pre[dp]
                    else:
                        xt = xpool.tile([P, 4, QTILE], bf16, tag="xt",
                                        name="xt")
                        nc.sync.dma_start(
                            xt[:], xT[:, 4 * dp : 4 * dp + 4, ns]
                        )
                    for u in range(4):
                        d = 4 * dp + u
                        st, sp = (d == 0), (d == DC - 1)
                        if half == 0:
                            for j in range(2):
                                for hc in range(2):
                                    nc.tensor.matmul(
                                        kps[j][hc][:],
                                        wk[j][:, d, hc * P : (hc + 1) * P],
                                        xt[:, u], start=st, stop=sp,
                                    )
                            for tci in range(4):
                                nc.tensor.matmul(
                                    vps[tci][:],
                                    xt[:, u, tci * P : (tci + 1) * P],
                                    wv[:, d, :], start=st, stop=sp,
                                )
                        else:
                            for j in range(4):
                                for hc in range(2):
                                    nc.tensor.matmul(
                                        qps[j][hc][:],
                                        wq[j][:, d, hc * P : (hc + 1) * P],
                                        xt[:, u], start=st, stop=sp,
                                    )
                # drains
                cos_t, sin_t = cos_a[:, ns], sin_a[:, ns]
                if half == 0:
                    for tci in range(4):
                        tg = n * 4 + tci
                        nc.scalar.copy(vA[:, tg, :], vps[tci][:])
                    rope_sets = [(j, kps[j]) for j in range(2)]
                else:
                    rope_sets = [(j, qps[j]) for j in range(4)]
                for j, ps in rope_sets:
                    t0 = tpool.tile([P, QTILE], bf16, tag="t0", name="t0")
                    t1 = tpool.tile([P, QTILE], bf16, tag="t1", name="t1")
                    nc.scalar.copy(t0[:], ps[0][:])
                    nc.vector.tensor_copy(t1[:], ps[1][:])
                    c0 = tpool.tile([P, QTILE], bf16, tag="c0", name="c0")
                    s0 = tpool.tile([P, QTILE], bf16, tag="s0", name="s0")
                    c1 = tpool.tile([P, QTILE], bf16, tag="c1", name="c1")
                    s1 = tpool.tile([P, QTILE], bf16, tag="s1", name="s1")
                    nc.vector.tensor_mul(c0[:], t0[:], cos_t)
                    nc.vector.tensor_mul(s0[:], t0[:], sin_t)
                    nc.vector.tensor_mul(c1[:], t1[:], cos_t)
                    nc.vector.tensor_mul(s1[:], t1[:], sin_t)
                    if half == 0:
                        nc.vector.tensor_sub(kT[j][:, 0, ns], c0[:], s1[:])
                        nc.vector.tensor_add(kT[j][:, 1, ns], c1[:], s0[:])
                    else:
                        pr, ab = j // 2, j % 2
                        for h2 in range(2):
                            cs = slice(h2 * ATILE, (h2 + 1) * ATILE)
                            nc.vector.tensor_sub(
                                qP[pr][2 * n + h2][:, 0, ab],
                                c0[:, cs], s1[:, cs],
                            )
                            nc.vector.tensor_add(
                                qP[pr][2 * n + h2][:, 1, ab],
                                c1[:, cs], s0[:, cs],
                            )

        ps1_cm.__exit__(None, None, None)
        tmp_cm.__exit__(None, None, None)
        xt_cm.__exit__(None, None, None)
        w_cm.__exit__(None, None, None)

        # ---------------- P2: attention ----------------
        owp_cm = tc.tile_pool(name="owp", bufs=1)
        owp = owp_cm.__enter__()
        ow_sb = []
        for j in range(2 * HEADS_PER_CORE):
            wt = owp.tile([P, D], bf16, tag=f"owp{j}", name=f"owp{j}")
            nc.sync.dma_start(wt[:], ow[j])
            ow_sb.append(wt)

        et_cm = tc.tile_pool(name="et", bufs=1)
        etp = et_cm.__enter__()
        eT = [etp.tile([P, 2, T], bf16, tag=f"eT{i}", name=f"eT{i}")
              for i in range(HEADS_PER_CORE)]

        sp_cm = tc.tile_pool(name="sp", bufs=3)
        spool = sp_cm.__enter__()
        np_cm = tc.tile_pool(name="np", bufs=3)
        npool = np_cm.__enter__()
        psL_cm = tc.tile_pool(name="psL", bufs=3, space="PSUM")
        psL = psL_cm.__enter__()
        psE_cm = tc.tile_pool(name="psE", bufs=1, space="PSUM")
        psE = psE_cm.__enter__()
        o3_cm = tc.tile_pool(name="o3", bufs=3)
        o3pool = o3_cm.__enter__()
        po_cm = tc.tile_pool(name="po", bufs=3, space="PSUM")
        popool = po_cm.__enter__()

        # attention and out-projection are emitted q-block-major and share
        # PSUM (3 L + 2 enc + 3 out-proj = 8 banks). The two q-heads of
        # each kv head are CONCATENATED along the free dim (qA|qB, 512
        # wide), so QK, tanh, exp, mask-mul, PV, and the denominator chain
        # each run once per k-tile for both heads at full 512-wide rates.
        # Denominator: DVE accumulate + GpSimd partition_all_reduce (no
        # ones-matmul, no PSUM bank, no 1-partition reciprocal).
        W2 = 2 * ATILE

        p3_ready = []

        def emit_p3_chunk():
            if not p3_ready:
                return
            tci = p3_ready.pop(0)
            ts_ = slice(tci * P, (tci + 1) * P)
            for nn in range(D // QTILE):
                nns = slice(nn * QTILE, (nn + 1) * QTILE)
                po = popool.tile([P, QTILE], f32, tag="po", name="po")
                for j in range(2 * HEADS_PER_CORE):
                    nc.tensor.matmul(
                        po[:], eT[j // 2][:, j % 2, ts_],
                        ow_sb[j][:, nns],
                        start=(j == 0),
                        stop=(j == 2 * HEADS_PER_CORE - 1),
                    )
                ob = o3pool.tile([P, QTILE], f32, tag="osb", name="osb")
                if nn % 2 == 0:
                    nc.vector.tensor_copy(ob[:], po[:])
                else:
                    nc.scalar.copy(ob[:], po[:])
                nc.sync.dma_start(out[ts_, nns], ob[:])

        def flush_norm(p):
            # reciprocal is emitted one pair-slot late: by now the
            # partition_all_reduce it waits on has finished, so the DVE
            # FIFO is not blocked; the eT muls run on GpSimd so any
            # residual wait lands on the idle engine, not DVE
            a, ecs, dent = p
            rb = npool.tile([P, W2], f32, tag="rb", name="rb")
            nc.vector.reciprocal_approx_fast(rb[:], dent[:])
            pr2, qs2 = a[1], slice(a[0] * ATILE, (a[0] + 1) * ATILE)
            for ab in range(2):
                h = 2 * pr2 + ab
                cs = slice(ab * ATILE, (ab + 1) * ATILE)
                for hc in range(2):
                    nc.vector.tensor_mul(
                        eT[h][:, hc, qs2], ecs[hc][:, cs], rb[:, cs]
                    )

        pending = None
        for at in range(NAT):
            kts = _kt_list(at)
            for pair in range(2):
                kvh = pair
                kTh = kT[kvh]
                enc_ps = [
                    psE.tile([P, W2], f32, tag=f"enc{hc}", name=f"enc{hc}")
                    for hc in range(2)
                ]
                acc = npool.tile([P, W2], f32, tag="acc", name="acc")
                for i, (kt, mi) in enumerate(kts):
                    st, sp = (i == 0), (i == len(kts) - 1)
                    L = psL.tile([P, W2], f32, tag="L", name="L")
                    for hc in range(2):
                        nc.tensor.matmul(
                            L[:], kTh[:, hc, kt * P : (kt + 1) * P],
                            qP[pair][at][:, hc], start=(hc == 0),
                            stop=(hc == 1),
                        )
                    tt = spool.tile([P, W2], f32, tag="tt", name="tt")
                    nc.scalar.activation(tt[:], L[:], AF.Tanh)
                    pp = spool.tile([P, W2], bf16, tag="pp", name="pp")
                    nc.scalar.activation(
                        pp[:], tt[:], AF.Exp, bias=bias_m50[:],
                        scale=SOFT_CAP,
                    )
                    pu = pp[:]
                    if mi is not None:
                        pm = spool.tile([P, W2], bf16, tag="pm", name="pm")
                        nc.vector.tensor_mul(pm[:], pp[:], masks_sb[:, mi])
                        pu = pm[:]
                    if st:
                        nc.vector.tensor_copy(acc[:], pu)
                    else:
                        nc.vector.tensor_add(acc[:], acc[:], pu)
                    base = kvh * H
                    for hc in range(2):
                        nc.tensor.matmul(
                            enc_ps[hc][:],
                            vA[:, kt, base + hc * P : base + (hc + 1) * P],
                            pu, start=st, stop=sp,
                        )
                # evacuate enc PSUM banks promptly (DVE copies) and start
                # the partition_all_reduce; the reciprocal + eT writes are
                # deferred one slot (see flush_norm)
                ecs = []
                for hc in range(2):
                    ec = npool.tile([P, W2], f32, tag=f"ec{hc}", name="ec")
                    nc.vector.tensor_copy(ec[:], enc_ps[hc][:])
                    ecs.append(ec)
                dent = npool.tile([P, W2], f32, tag="dn", name="dent")
                nc.gpsimd.partition_all_reduce(
                    dent[:], acc[:], P, bass_isa.ReduceOp.add
                )
                if pending is not None:
                    flush_norm(pending)
                    if pending[0][1] == 1:
                        a0 = pending[0][0]
                        p3_ready.extend([2 * a0, 2 * a0 + 1])
                    emit_p3_chunk()
                pending = ((at, pair), ecs, dent)
        flush_norm(pending)
        p3_ready.extend([2 * (NAT - 1), 2 * (NAT - 1) + 1])
        while p3_ready:
            emit_p3_chunk()

        po_cm.__exit__(None, None, None)
        o3_cm.__exit__(None, None, None)
        psE_cm.__exit__(None, None, None)
        psL_cm.__exit__(None, None, None)
        np_cm.__exit__(None, None, None)
        sp_cm.__exit__(None, None, None)
        et_cm.__exit__(None, None, None)
        owp_cm.__exit__(None, None, None)
        pers_cm.__exit__(None, None, None)

    nc.finalize()
    return nc


def _install_axon_hooks_shim():
    """Provide antenv.axon_hooks if the image lacks it (NTFF profiling)."""
    import types

    try:
        import antenv.axon_hooks  # noqa: F401

        return
    except ImportError:
        pass
    hook = None
    try:
        from trn_agent_boot.trn_boot import _ntff_profile_via_ctypes

        hook = _ntff_profile_via_ctypes("/opt/axon/libaxon_pjrt.so")
    except Exception:
        hook = None
    mod = types.ModuleType("antenv.axon_hooks")
    _h = [hook]
    mod.get_axon_ntff_profile_hook = lambda: _h[0]

    def _set(h):
        _h[0] = h

    mod.set_axon_ntff_profile_hook = _set
    sys.modules["antenv.axon_hooks"] = mod
    try:
        import antenv

        antenv.axon_hooks = mod
    except ImportError:
        pass


def _install_neff_cache():
    """Cache walrus-compiled NEFFs by BIR hash (compiles are minutes-long)."""
    import hashlib
    import shutil

    import concourse.bass2jax as b2j

    if getattr(b2j, "_ant_neff_cache_installed", False):
        return
    orig = b2j.compile_bir_kernel

    def cached(bir_json, tmpdir, neff_name="file.neff"):
        cdir = os.environ.get("NEFF_CACHE_DIR", "/tmp/neff_cache")
        os.makedirs(cdir, exist_ok=True)
        h = hashlib.sha256(bir_json).hexdigest()[:32]
        cpath = os.path.join(cdir, f"{h}.neff")
        if os.path.exists(cpath):
            dst = os.path.join(tmpdir, "sg00")
            os.makedirs(dst, exist_ok=True)
            dstf = os.path.join(dst, neff_name)
            shutil.copyfile(cpath, dstf)
            return dstf
        r = orig(bir_json, tmpdir, neff_name=neff_name)
        try:
            shutil.copyfile(r, cpath)
        except OSError:
            pass
        return r

    b2j.compile_bir_kernel = cached
    b2j._ant_neff_cache_installed = True


def _host_inputs(x, segment_pos, q_w, kv_w, out_w):
    """Per-core input maps (bf16 host-side prep)."""
    import ml_dtypes

    bf = ml_dtypes.bfloat16
    QS = SCALE / SOFT_CAP

    def _wlayout(w):
        # [nh, D, H] -> [nh, P, DC, H]: per-partition contiguous spans
        return np.ascontiguousarray(
            w.reshape(-1, DC, P, w.shape[-1]).transpose(0, 2, 1, 3)
        ).astype(bf)

    ropes = []
    for b in range(B):
        pos = segment_pos[b].astype(np.float32)
        fraction = 2.0 * np.arange(P, dtype=np.float32) / H
        timescale = BASE_FREQ**fraction
        ang = pos[None, :] / timescale[:, None]          # [128, T]
        r = np.stack([np.cos(ang), np.sin(ang)])
        ropes.append(
            np.ascontiguousarray(r.transpose(1, 0, 2)).astype(bf)
        )
    m1 = _make_masks()
    masks = np.ascontiguousarray(
        np.concatenate([m1, m1], axis=2).transpose(1, 0, 2)
    ).astype(bf)

    xTs = []
    for b in range(B):
        xt = np.ascontiguousarray(
            x[b].T.reshape(DC, P, T).transpose(1, 0, 2)
        ).astype(bf)
        xTs.append(xt)

    in_maps = []
    for core in range(8):
        b, g = core // 4, core % 4
        qws = _wlayout(q_w[4 * g : 4 * g + 4] * QS)
        kws = _wlayout(kv_w[0, 2 * g : 2 * g + 2])
        # pack both v heads along H: [P, DC, 2H]
        vss = _wlayout(kv_w[1, 2 * g : 2 * g + 2])   # [2, P, DC, H]
        vwp = np.ascontiguousarray(
            np.concatenate([vss[0], vss[1]], axis=-1)
        )
        ows = np.ascontiguousarray(
            out_w[4 * g : 4 * g + 4].reshape(2 * HEADS_PER_CORE, P, D)
        ).astype(bf)
        in_maps.append(
            {
                "xT": xTs[b],
                "qw": qws,
                "kw": kws,
                "vw": vwp,
                "ow": ows,
                "rope": ropes[b],
                "msk": masks,
            }
        )
    return in_maps


def kernel(x, segment_pos, attn_mask, q_w, kv_w, out_w):
    global LAST_RESULTS
    from concourse.bass_utils import run_bass_kernel_spmd

    _install_axon_hooks_shim()
    _install_neff_cache()

    x = np.asarray(x, np.float32)
    segment_pos = np.asarray(segment_pos, np.int32)
    q_w = np.asarray(q_w, np.float32)
    kv_w = np.asarray(kv_w, np.float32)
    out_w = np.asarray(out_w, np.float32)

    key = "main"
    if key not in _NC_CACHE:
        _NC_CACHE[key] = _build_nc()
    nc = _NC_CACHE[key]

    in_maps = _host_inputs(x, segment_pos, q_w, kv_w, out_w)
    res = run_bass_kernel_spmd(nc, in_maps, core_ids=list(range(8)))
    LAST_RESULTS = res

    outv = np.zeros((B, T, D), np.float32)
    for core in range(8):
        outv[core // 4] += res.results[core]["out"]
    return outv


# revision 8
# speedup vs baseline: 1.0027x; 1.0027x over previous
"""Trainium2 Bass kernel V2: GQA sliding-window attention, SBUF-resident bf16.

Problem: B=2, T=2048, D=3584, N=16 q-heads, K=8 kv-heads, H=256,
sliding window 1024, causal, soft-cap 50, query scale 0.0625, RoPE.

Sharding: 8 cores = 2 (batch) x 4 (head groups); each core: 4 q-heads,
2 kv-heads. Host sums the 4 partial out-projections per batch.

V2 design vs baseline:
  - All weights/intermediates bf16 (halves DMA + SBUF, full PE rate).
  - Zero DRAM round-trips: kT/qT/v/eT live in SBUF end-to-end.
  - V projected directly into [t, h] layout (stationary = x chunk), so
    no PE transposes and no copies.
  - SCALE/SOFT_CAP folded into q_w on host; RoPE drain = ACT copy
    (psum->bf16) + 6 bf16 DVE ops (4x mode) writing kT/qT in place.
  - Attention interleaves the two q-heads of each kv head to keep PE fed
    while the softmax chain (ACT tanh/exp, DVE mask-mul) runs.
  - Out-proj reads eT straight from SBUF, accumulating 8 matmuls in PSUM.
"""

import os
import sys

sys.path.insert(0, "/opt/trn_rl_repo")

import numpy as np

B, T, D = 2, 2048, 3584
NQ, NKV, H = 16, 8, 256
P = 128
DC = D // P                 # 28 contraction chunks
HEADS_PER_CORE = 4
KV_PER_CORE = 2
SOFT_CAP = 50.0
SCALE = 0.0625
WINDOW = 1024
BASE_FREQ = 10000.0
QTILE = 512
NQT = T // QTILE            # 4
ATILE = 256
NAT = T // ATILE            # 8
NKT = T // P                # 16

_NC_CACHE = {}
LAST_RESULTS = None


def _kt_list(at):
    """Valid k-tiles for q-block at (ATILE wide); mask index None = full."""
    Q0 = at * ATILE
    out = []
    for kt in range(NKT):
        K0 = kt * P
        if K0 > Q0 + ATILE - 1:
            continue
        if K0 + P - 1 <= Q0 - WINDOW:
            continue
        rel = K0 - Q0
        if rel >= 0:
            out.append((kt, rel // P))
        else:
            w = Q0 - K0 - WINDOW
            if -ATILE < w <= 0:
                out.append((kt, 2 + (-w) // P))
            else:
                out.append((kt, None))
    return out


def _make_masks():
    m = np.zeros((4, P, ATILE), np.float32)
    i = np.arange(P)[:, None]
    j = np.arange(ATILE)[None, :]
    for r in range(2):           # diag: allowed iff i <= j - rel
        m[r] = np.where(i <= j - r * P, 1.0, 0.0)
    for wi in range(2):          # window: allowed iff i > j - wi*128
        m[2 + wi] = np.where(i > j - wi * P, 1.0, 0.0)
    return m


def _build_nc():
    import concourse.bacc as bacc
    import concourse.mybir as mybir
    import concourse.tile as tile
    from concourse import bass_isa

    f32 = mybir.dt.float32
    bf16 = mybir.dt.bfloat16
    AF = mybir.ActivationFunctionType

    nc = bacc.Bacc()
    xT = nc.dram_tensor("xT", (P, DC, T), bf16, kind="ExternalInput")
    qw = nc.dram_tensor("qw", (HEADS_PER_CORE, P, DC, H), bf16,
                        kind="ExternalInput")
    kw = nc.dram_tensor("kw", (KV_PER_CORE, P, DC, H), bf16,
                        kind="ExternalInput")
    vw = nc.dram_tensor("vw", (P, DC, 2 * H), bf16, kind="ExternalInput")
    ow = nc.dram_tensor("ow", (2 * HEADS_PER_CORE, P, D), bf16,
                        kind="ExternalInput")
    rope = nc.dram_tensor("rope", (P, 2, T), bf16, kind="ExternalInput")
    msk = nc.dram_tensor("msk", (P, 4, 2 * ATILE), bf16,
                         kind="ExternalInput")
    out = nc.dram_tensor("out", (T, D), f32, kind="ExternalOutput")

    with tile.TileContext(nc) as tc:
        pers_cm = tc.tile_pool(name="pers", bufs=1)
        pers = pers_cm.__enter__()

        # Persistent SBUF state
        kT = [pers.tile([P, 2, T], bf16, tag=f"kT{i}", name=f"kT{i}")
              for i in range(KV_PER_CORE)]
        # q stored pair-packed per at-block: [hc, head-in-pair, ATILE] so
        # one 512-wide moving operand covers both heads of a kv pair, and
        # per-at tiles keep attention deps slice-accurate
        qP = [
            [pers.tile([P, 2, 2, ATILE], bf16, tag=f"qP{i}_{a}",
                       name=f"qP{i}_{a}") for a in range(NAT)]
            for i in range(KV_PER_CORE)
        ]
        vA = pers.tile([P, NKT, 2 * H], bf16, tag="vA", name="vA")
        masks_sb = pers.tile([P, 4, 2 * ATILE], bf16, tag="msk",
                             name="masks_sb")
        bias_m50 = pers.tile([P, 1], f32, tag="b50", name="bias_m50")
        nc.vector.memset(bias_m50[:], -SOFT_CAP)
        # warm the GpSimd ucode library during P1 so the first real
        # partition_all_reduce in attention doesn't stall ~10us on I$ load
        gwarm = pers.tile([P, 1], f32, tag="gw", name="gwarm")
        nc.gpsimd.partition_all_reduce(gwarm[:], bias_m50[:], P,
                                       bass_isa.ReduceOp.add)

        # ---------------- P1: projections + RoPE ----------------
        w_cm = tc.tile_pool(name="w", bufs=1)
        wpool = w_cm.__enter__()
        # rope table only lives through P1 — keep it in the P1 pool
        rope_sb = wpool.tile([P, 2, T], bf16, tag="rope", name="rope_sb")
        cos_a = rope_sb[:, 0]
        sin_a = rope_sb[:, 1]
        xt_cm = tc.tile_pool(name="xt", bufs=5)
        xpool = xt_cm.__enter__()
        tmp_cm = tc.tile_pool(name="tmp", bufs=2)
        tpool = tmp_cm.__enter__()
        ps1_cm = tc.tile_pool(name="ps1", bufs=1, space="PSUM")
        ps1 = ps1_cm.__enter__()

        for half in range(2):  # 0: k0,k1 + v(packed)   1: q0..q3
            # DMA issue order matters: every dma_start serializes on the
            # sync engine (~0.6us each), so issue the first-needed chunks
            # first: leading halves of the weights, then the first x tiles,
            # then the weight tails (and rope/masks, needed ~40us in).
            CSPLIT = 14

            if half == 0:
                # interleave weight-chunk and x-tile issue in contraction
                # order (geometric chunks): PE consumes ~380KB per d-group,
                # so arrival order must track the d loop or PE starves
                wk = [wpool.tile([P, DC, H], bf16, tag=f"w{j}",
                                 name=f"wk{j}") for j in range(2)]
                wv = wpool.tile([P, DC, 2 * H], bf16, tag="w2", name="wv")
                xt_pre = []

                def _xt_dma(d4):
                    xt = xpool.tile([P, 4, QTILE], bf16, tag="xt",
                                    name="xt")
                    nc.sync.dma_start(
                        xt[:], xT[:, 4 * d4 : 4 * d4 + 4, 0:QTILE]
                    )
                    xt_pre.append(xt)

                bounds = [0, 2, 6, 14, DC]
                for ci in range(4):
                    cs = slice(bounds[ci], bounds[ci + 1])
                    for j in range(2):
                        nc.sync.dma_start(wk[j][:, cs], kw[j][:, cs])
                    nc.sync.dma_start(wv[:, cs], vw[:, cs])
                    _xt_dma(ci)
                nc.sync.dma_start(rope_sb[:], rope[:])
                nc.sync.dma_start(masks_sb[:], msk[:])
            else:
                wq = [wq0]
                for j in range(1, 4):
                    wt = wpool.tile([P, DC, H], bf16, tag=f"w{j}",
                                    name=f"wq{j}")
                    nc.sync.dma_start(wt[:], qw[j])
                    wq.append(wt)

            for n in range(NQT):
                ns = slice(n * QTILE, (n + 1) * QTILE)
                if half == 0 and n == 2:
                    # qw0 prefetch on its own slot mid-half-0: no WAR gate,
                    # and the x-stream has DMA slack by now
                    wq0 = wpool.tile([P, DC, H], bf16, tag="w4", name="wq0")
                    nc.sync.dma_start(wq0[:], qw[0])
                if half == 0:
                    # k psums: tags t0..t3; v psums (per t-chunk): t4..t7
                    kps = [
                        [ps1.tile([P, QTILE], f32, tag=f"t{2 * j + hc}",
                                  name=f"kps{j}{hc}") for hc in range(2)]
                        for j in range(2)
                    ]
                    vps = [ps1.tile([P, 2 * H], f32, tag=f"t{4 + tci}",
                                    name=f"vps{tci}") for tci in range(4)]
                else:
                    qps = [
                        [ps1.tile([P, QTILE], f32, tag=f"t{2 * j + hc}",
                                  name=f"qps{j}{hc}") for hc in range(2)]
                        for j in range(4)
                    ]
                xts = []
                for dp in range(DC // 4):
                    if half == 0 and n == 0 and dp < 4:
                        xt = xt_pre[dp]
                    else:
                        xt = xpool.tile([P, 4, QTILE], bf16, tag="xt",
                                        name="xt")
                        nc.sync.dma_start(
                            xt[:], xT[:, 4 * dp : 4 * dp + 4, ns]
                        )
                    xts.append(xt)
                    for u in range(4):
                        d = 4 * dp + u
                        st, sp = (d == 0), (d == DC - 1)
                        if half == 0:
                            for j in range(2):
                                for hc in range(2):
                                    nc.tensor.matmul(
                                        kps[j][hc][:],
                                        wk[j][:, d, hc * P : (hc + 1) * P],
                                        xt[:, u], start=st, stop=sp,
                                    )
                            for tci in range(4):
                                nc.tensor.matmul(
                                    vps[tci][:],
                                    xt[:, u, tci * P : (tci + 1) * P],
                                    wv[:, d, :], start=st, stop=sp,
                                )
                        else:
                            for j in range(4):
                                for hc in range(2):
                                    nc.tensor.matmul(
                                        qps[j][hc][:],
                                        wq[j][:, d, hc * P : (hc + 1) * P],
                                        xt[:, u], start=st, stop=sp,
                                    )
                # drains
                cos_t, sin_t = cos_a[:, ns], sin_a[:, ns]
                if half == 0:
                    for tci in range(4):
                        tg = n * 4 + tci
                        nc.scalar.copy(vA[:, tg, :], vps[tci][:])
                    rope_sets = [(j, kps[j]) for j in range(2)]
                else:
                    rope_sets = [(j, qps[j]) for j in range(4)]
                for j, ps in rope_sets:
                    t0 = tpool.tile([P, QTILE], bf16, tag="t0", name="t0")
                    t1 = tpool.tile([P, QTILE], bf16, tag="t1", name="t1")
                    nc.scalar.copy(t0[:], ps[0][:])
                    nc.scalar.copy(t1[:], ps[1][:])
                    c0 = tpool.tile([P, QTILE], bf16, tag="c0", name="c0")
                    s0 = tpool.tile([P, QTILE], bf16, tag="s0", name="s0")
                    c1 = tpool.tile([P, QTILE], bf16, tag="c1", name="c1")
                    s1 = tpool.tile([P, QTILE], bf16, tag="s1", name="s1")
                    nc.vector.tensor_mul(c0[:], t0[:], cos_t)
                    nc.vector.tensor_mul(s0[:], t0[:], sin_t)
                    nc.vector.tensor_mul(c1[:], t1[:], cos_t)
                    nc.vector.tensor_mul(s1[:], t1[:], sin_t)
                    if half == 0:
                        nc.vector.tensor_sub(kT[j][:, 0, ns], c0[:], s1[:])
                        nc.vector.tensor_add(kT[j][:, 1, ns], c1[:], s0[:])
                    else:
                        pr, ab = j // 2, j % 2
                        for h2 in range(2):
                            cs = slice(h2 * ATILE, (h2 + 1) * ATILE)
                            nc.vector.tensor_sub(
                                qP[pr][2 * n + h2][:, 0, ab],
                                c0[:, cs], s1[:, cs],
                            )
                            nc.vector.tensor_add(
                                qP[pr][2 * n + h2][:, 1, ab],
                                c1[:, cs], s0[:, cs],
                            )

        ps1_cm.__exit__(None, None, None)
        tmp_cm.__exit__(None, None, None)
        xt_cm.__exit__(None, None, None)
        w_cm.__exit__(None, None, None)

        # ---------------- P2: attention ----------------
        owp_cm = tc.tile_pool(name="owp", bufs=1)
        owp = owp_cm.__enter__()
        ow_sb = []
        for j in range(2 * HEADS_PER_CORE):
            wt = owp.tile([P, D], bf16, tag=f"owp{j}", name=f"owp{j}")
            nc.sync.dma_start(wt[:], ow[j])
            ow_sb.append(wt)

        et_cm = tc.tile_pool(name="et", bufs=1)
        etp = et_cm.__enter__()
        eT = [etp.tile([P, 2, T], bf16, tag=f"eT{i}", name=f"eT{i}")
              for i in range(HEADS_PER_CORE)]

        sp_cm = tc.tile_pool(name="sp", bufs=3)
        spool = sp_cm.__enter__()
        np_cm = tc.tile_pool(name="np", bufs=3)
        npool = np_cm.__enter__()
        psL_cm = tc.tile_pool(name="psL", bufs=3, space="PSUM")
        psL = psL_cm.__enter__()
        psE_cm = tc.tile_pool(name="psE", bufs=1, space="PSUM")
        psE = psE_cm.__enter__()
        o3_cm = tc.tile_pool(name="o3", bufs=3)
        o3pool = o3_cm.__enter__()
        po_cm = tc.tile_pool(name="po", bufs=3, space="PSUM")
        popool = po_cm.__enter__()

        # attention and out-projection are emitted q-block-major and share
        # PSUM (3 L + 2 enc + 3 out-proj = 8 banks). The two q-heads of
        # each kv head are CONCATENATED along the free dim (qA|qB, 512
        # wide), so QK, tanh, exp, mask-mul, PV, and the denominator chain
        # each run once per k-tile for both heads at full 512-wide rates.
        # Denominator: DVE accumulate + GpSimd partition_all_reduce (no
        # ones-matmul, no PSUM bank, no 1-partition reciprocal).
        W2 = 2 * ATILE

        p3_ready = []

        def emit_p3_chunk():
            if not p3_ready:
                return
            tci = p3_ready.pop(0)
            ts_ = slice(tci * P, (tci + 1) * P)
            for nn in range(D // QTILE):
                nns = slice(nn * QTILE, (nn + 1) * QTILE)
                po = popool.tile([P, QTILE], f32, tag="po", name="po")
                for j in range(2 * HEADS_PER_CORE):
                    nc.tensor.matmul(
                        po[:], eT[j // 2][:, j % 2, ts_],
                        ow_sb[j][:, nns],
                        start=(j == 0),
                        stop=(j == 2 * HEADS_PER_CORE - 1),
                    )
                ob = o3pool.tile([P, QTILE], f32, tag="osb", name="osb")
                if nn % 2 == 0:
                    nc.vector.tensor_copy(ob[:], po[:])
                else:
                    nc.scalar.copy(ob[:], po[:])
                nc.sync.dma_start(out[ts_, nns], ob[:])

        def flush_norm(p):
            # reciprocal is emitted one pair-slot late: by now the
            # partition_all_reduce it waits on has finished, so the DVE
            # FIFO is not blocked; the eT muls run on GpSimd so any
            # residual wait lands on the idle engine, not DVE
            a, ecs, dent = p
            rb = npool.tile([P, W2], f32, tag="rb", name="rb")
            nc.vector.reciprocal_approx_fast(rb[:], dent[:])
            pr2, qs2 = a[1], slice(a[0] * ATILE, (a[0] + 1) * ATILE)
            for ab in range(2):
                h = 2 * pr2 + ab
                cs = slice(ab * ATILE, (ab + 1) * ATILE)
                for hc in range(2):
                    nc.vector.tensor_mul(
                        eT[h][:, hc, qs2], ecs[hc][:, cs], rb[:, cs]
                    )

        pending = None
        for at in range(NAT):
            kts = _kt_list(at)
            for pair in range(2):
                kvh = pair
                kTh = kT[kvh]
                enc_ps = [
                    psE.tile([P, W2], f32, tag=f"enc{hc}", name=f"enc{hc}")
                    for hc in range(2)
                ]
                acc = npool.tile([P, W2], f32, tag="acc", name="acc")
                for i, (kt, mi) in enumerate(kts):
                    st, sp = (i == 0), (i == len(kts) - 1)
                    L = psL.tile([P, W2], f32, tag="L", name="L")
                    for hc in range(2):
                        nc.tensor.matmul(
                            L[:], kTh[:, hc, kt * P : (kt + 1) * P],
                            qP[pair][at][:, hc], start=(hc == 0),
                            stop=(hc == 1),
                        )
                    tt = spool.tile([P, W2], f32, tag="tt", name="tt")
                    nc.scalar.activation(tt[:], L[:], AF.Tanh)
                    pp = spool.tile([P, W2], bf16, tag="pp", name="pp")
                    nc.scalar.activation(
                        pp[:], tt[:], AF.Exp, bias=bias_m50[:],
                        scale=SOFT_CAP,
                    )
                    pu = pp[:]
                    if mi is not None:
                        pm = spool.tile([P, W2], bf16, tag="pm", name="pm")
                        nc.vector.tensor_mul(pm[:], pp[:], masks_sb[:, mi])
                        pu = pm[:]
                    if st:
                        nc.vector.tensor_copy(acc[:], pu)
                    else:
                        nc.vector.tensor_add(acc[:], acc[:], pu)
                    base = kvh * H
                    for hc in range(2):
                        nc.tensor.matmul(
                            enc_ps[hc][:],
                            vA[:, kt, base + hc * P : base + (hc + 1) * P],
                            pu, start=st, stop=sp,
                        )
                # evacuate enc PSUM banks promptly (DVE copies) and start
                # the partition_all_reduce; the reciprocal + eT writes are
                # deferred one slot (see flush_norm)
                ecs = []
                for hc in range(2):
                    ec = npool.tile([P, W2], f32, tag=f"ec{hc}", name="ec")
                    nc.vector.tensor_copy(ec[:], enc_ps[hc][:])
                    ecs.append(ec)
                dent = npool.tile([P, W2], f32, tag="dn", name="dent")
                nc.gpsimd.partition_all_reduce(
                    dent[:], acc[:], P, bass_isa.ReduceOp.add
                )
                if pending is not None:
                    flush_norm(pending)
                    if pending[0][1] == 1:
                        a0 = pending[0][0]
                        p3_ready.extend([2 * a0, 2 * a0 + 1])
                    emit_p3_chunk()
                pending = ((at, pair), ecs, dent)
        flush_norm(pending)
        p3_ready.extend([2 * (NAT - 1), 2 * (NAT - 1) + 1])
        while p3_ready:
            emit_p3_chunk()

        po_cm.__exit__(None, None, None)
        o3_cm.__exit__(None, None, None)
        psE_cm.__exit__(None, None, None)
        psL_cm.__exit__(None, None, None)
        np_cm.__exit__(None, None, None)
        sp_cm.__exit__(None, None, None)
        et_cm.__exit__(None, None, None)
        owp_cm.__exit__(None, None, None)
        pers_cm.__exit__(None, None, None)

    nc.finalize()
    return nc


def _install_axon_hooks_shim():
    """Provide antenv.axon_hooks if the image lacks it (NTFF profiling)."""
    import types

    try:
        import antenv.axon_hooks  # noqa: F401

        return
    except ImportError:
        pass
    hook = None
    try:
        from trn_agent_boot.trn_boot import _ntff_profile_via_ctypes

        hook = _ntff_profile_via_ctypes("/opt/axon/libaxon_pjrt.so")
    except Exception:
        hook = None
    mod = types.ModuleType("antenv.axon_hooks")
    _h = [hook]
    mod.get_axon_ntff_profile_hook = lambda: _h[0]

    def _set(h):
        _h[0] = h

    mod.set_axon_ntff_profile_hook = _set
    sys.modules["antenv.axon_hooks"] = mod
    try:
        import antenv

        antenv.axon_hooks = mod
    except ImportError:
        pass


def _install_neff_cache():
    """Cache walrus-compiled NEFFs by BIR hash (compiles are minutes-long)."""
    import hashlib
    import shutil

    import concourse.bass2jax as b2j

    if getattr(b2j, "_ant_neff_cache_installed", False):
        return
    orig = b2j.compile_bir_kernel

    def cached(bir_json, tmpdir, neff_name="file.neff"):
        cdir = os.environ.get("NEFF_CACHE_DIR", "/tmp/neff_cache")
        os.makedirs(cdir, exist_ok=True)
        h = hashlib.sha256(bir_json).hexdigest()[:32]
        cpath = os.path.join(cdir, f"{h}.neff")
        if os.path.exists(cpath):
            dst = os.path.join(tmpdir, "sg00")
            os.makedirs(dst, exist_ok=True)
            dstf = os.path.join(dst, neff_name)
            shutil.copyfile(cpath, dstf)
            return dstf
        r = orig(bir_json, tmpdir, neff_name=neff_name)
        try:
            shutil.copyfile(r, cpath)
        except OSError:
            pass
        return r

    b2j.compile_bir_kernel = cached
    b2j._ant_neff_cache_installed = True


def _host_inputs(x, segment_pos, q_w, kv_w, out_w):
    """Per-core input maps (bf16 host-side prep)."""
    import ml_dtypes

    bf = ml_dtypes.bfloat16
    QS = SCALE / SOFT_CAP

    def _wlayout(w):
        # [nh, D, H] -> [nh, P, DC, H]: per-partition contiguous spans
        return np.ascontiguousarray(
            w.reshape(-1, DC, P, w.shape[-1]).transpose(0, 2, 1, 3)
        ).astype(bf)

    ropes = []
    for b in range(B):
        pos = segment_pos[b].astype(np.float32)
        fraction = 2.0 * np.arange(P, dtype=np.float32) / H
        timescale = BASE_FREQ**fraction
        ang = pos[None, :] / timescale[:, None]          # [128, T]
        r = np.stack([np.cos(ang), np.sin(ang)])
        ropes.append(
            np.ascontiguousarray(r.transpose(1, 0, 2)).astype(bf)
        )
    m1 = _make_masks()
    masks = np.ascontiguousarray(
        np.concatenate([m1, m1], axis=2).transpose(1, 0, 2)
    ).astype(bf)

    xTs = []
    for b in range(B):
        xt = np.ascontiguousarray(
            x[b].T.reshape(DC, P, T).transpose(1, 0, 2)
        ).astype(bf)
        xTs.append(xt)

    in_maps = []
    for core in range(8):
        b, g = core // 4, core % 4
        qws = _wlayout(q_w[4 * g : 4 * g + 4] * QS)
        kws = _wlayout(kv_w[0, 2 * g : 2 * g + 2])
        # pack both v heads along H: [P, DC, 2H]
        vss = _wlayout(kv_w[1, 2 * g : 2 * g + 2])   # [2, P, DC, H]
        vwp = np.ascontiguousarray(
            np.concatenate([vss[0], vss[1]], axis=-1)
        )
        ows = np.ascontiguousarray(
            out_w[4 * g : 4 * g + 4].reshape(2 * HEADS_PER_CORE, P, D)
        ).astype(bf)
        in_maps.append(
            {
                "xT": xTs[b],
                "qw": qws,
                "kw": kws,
                "vw": vwp,
                "ow": ows,
                "rope": ropes[b],
                "msk": masks,
            }
        )
    return in_maps


def kernel(x, segment_pos, attn_mask, q_w, kv_w, out_w):
    global LAST_RESULTS
    from concourse.bass_utils import run_bass_kernel_spmd

    _install_axon_hooks_shim()
    _install_neff_cache()

    x = np.asarray(x, np.float32)
    segment_pos = np.asarray(segment_pos, np.int32)
    q_w = np.asarray(q_w, np.float32)
    kv_w = np.asarray(kv_w, np.float32)
    out_w = np.asarray(out_w, np.float32)

    key = "main"
    if key not in _NC_CACHE:
        _NC_CACHE[key] = _build_nc()
    nc = _NC_CACHE[key]

    in_maps = _host_inputs(x, segment_pos, q_w, kv_w, out_w)
    res = run_bass_kernel_spmd(nc, in_maps, core_ids=list(range(8)))
    LAST_RESULTS = res

    outv = np.zeros((B, T, D), np.float32)
    for core in range(8):
        outv[core // 4] += res.results[core]["out"]
    return outv
